# revision 1
# baseline (speedup 1.0000x reference)
"""Trainium2 Bass kernel: NF4 (bitsandbytes-style) dequant + linear.

y = x @ dequant(weight_q, absmax).T + bias

x:        [4, 2048, 4096] f32
weight_q: [11008, 4096] int32 (values 0..15, NF4 codes)
absmax:   [11008, 64] f32 (per-64-block scales)
bias:     [11008] f32
out:      [4, 2048, 11008] f32

Sharding: column-parallel over out_features across 8 cores (1376 each).
The kernel computes in bf16 (f32 PSUM accumulation); x is host-prepped
into the kernel's internal operand format: transposed to [4096, 8192]
(contraction dim on SBUF partitions) and rounded to bf16 once. Weight
indices ship as 0/1 bit/selector planes (index layout prep) so on-device
dequant is a short select tree:
  pair_j  = b0 ? c[2j+1] : c[2j]      (scalar-engine affine leaf)
  quad_j  = pair_2j <-m1- pair_2j+1   (DVE predicated overwrite)
  code    = quad_0 <-ind_j- quad_j
  w       = code * absmax             (bf16)
Matmuls run kt-outer / o-chunk-inner so each stationary (x) tile feeds 3
matmuls, and the token loop is phased (first token tiles touch one
o-chunk at a time) so PE weight demand tracks dequant progress instead
of stalling on the full weight set.
"""

import numpy as np
import ml_dtypes

import concourse.bacc as bacc
import concourse.mybir as mybir
import concourse.tile as tile
from concourse.alu_op_type import AluOpType
from concourse.bass_utils import run_bass_kernel_spmd

DT = mybir.dt

NF4 = [
    -1.0, -0.6961928009986877, -0.5250730514526367, -0.39491748809814453,
    -0.28444138169288635, -0.18477343022823334, -0.09105003625154495, 0.0,
    0.07958029955625534, 0.16093020141124725, 0.24611230194568634,
    0.33791524171829224, 0.44070982933044434, 0.5626170039176941,
    0.7229568362236023, 1.0]

P = 128
IN_F = 4096
OUT_F = 11008
N_CORES = 8
O_LOC = OUT_F // N_CORES          # 1376 out features per core
S_TOT = 4 * 2048                  # 8192 tokens
KT = IN_F // P                    # 32 contraction tiles
SP = 256                          # tokens per x macro tile (2 psum tiles)
NSP = S_TOT // SP                 # 32 x macro tiles
O_CHUNKS = [(1024, 352), (0, 512), (512, 512)]
PHASE_SPS = 14                    # x macro tiles in each phased (A/B/C) pass

_CACHE = {}


def _dequant_chunk(nc, dq, wpool, planes, sc_t, oi, kt, osz):
    """Select-tree dequant of one [128, osz] chunk. planes = (b0 bf16,
    m1/i1/i2/i3 uint8) SBUF tiles. Returns persistent bf16 weight tile."""
    b0, m1, i1, i2, i3 = planes  # b0 is a tile, masks are APs into mpack

    def leaf(out_t, lo, hi, eng):
        if eng == "act":
            nc.scalar.activation(out_t[:], b0[:],
                                 mybir.ActivationFunctionType.Copy,
                                 bias=float(NF4[lo]),
                                 scale=float(NF4[hi] - NF4[lo]))
        else:
            e = nc.gpsimd if eng == "pool" else nc.vector
            e.tensor_scalar(out_t[:], b0[:], NF4[hi] - NF4[lo], NF4[lo],
                            AluOpType.mult, AluOpType.add)

    t4 = dq.tile([P, osz], DT.bfloat16, tag="t4")
    qo = dq.tile([P, osz], DT.bfloat16, tag="qo0")
    leaf(t4, 0, 1, "act")
    leaf(qo, 2, 3, "act")
    nc.vector.copy_predicated(t4[:], m1, qo[:])

    quads = []
    for j, ind in ((1, i1), (2, i2), (3, i3)):
        qe = dq.tile([P, osz], DT.bfloat16, tag=f"qe{j}")
        qo_j = dq.tile([P, osz], DT.bfloat16, tag=f"qo{j}")
        leaf(qe, 4 * j, 4 * j + 1, "act")
        leaf(qo_j, 4 * j + 2, 4 * j + 3, "act" if j == 2 else "dve")
        nc.vector.copy_predicated(qe[:], m1, qo_j[:])
        quads.append((qe, ind))

    for qe, ind in quads:
        nc.vector.copy_predicated(t4[:], ind, qe[:])

    w_t = wpool.tile([P, osz], DT.bfloat16, tag=f"w_{oi}_{kt}")
    nc.vector.tensor_tensor(w_t[:], t4[:], sc_t[:], AluOpType.mult)
    return w_t


def _build():
    nc = bacc.Bacc()
    xT = nc.dram_tensor("xT", [IN_F, S_TOT], DT.bfloat16, kind="ExternalInput")
    b0_d = nc.dram_tensor("b0", [IN_F, O_LOC], DT.bfloat16,
                          kind="ExternalInput")
    mpack_d = nc.dram_tensor("mpack", [KT, P, 4, O_LOC], DT.uint8,
                             kind="ExternalInput")
    scale = nc.dram_tensor("scale", [KT, P, O_LOC], DT.bfloat16, kind="ExternalInput")
    biasb = nc.dram_tensor("biasb", [1, O_LOC], DT.bfloat16, kind="ExternalInput")
    y = nc.dram_tensor("y", [S_TOT, O_LOC], DT.float32, kind="ExternalOutput")

    with tile.TileContext(nc) as tc:
        with (
            tc.tile_pool(name="w", bufs=1) as wpool,
            tc.tile_pool(name="dqp", bufs=3) as dqp,
            tc.tile_pool(name="dq", bufs=2) as dq,
            tc.tile_pool(name="x", bufs=3) as xp,
            tc.tile_pool(name="o", bufs=4) as op,
            tc.tile_pool(name="ps", bufs=8, space="PSUM") as psp,
            tc.tile_pool(name="c", bufs=1) as cst,
        ):
            bias_t = cst.tile([1, O_LOC], DT.bfloat16)
            nc.sync.dma_start(out=bias_t[:], in_=biasb[:])
            biasw = cst.tile([P, O_LOC], DT.float32)
            nc.gpsimd.dma_start(out=biasw[:],
                                in_=biasb[0, :].partition_broadcast(P))
            ones_t = cst.tile([1, P], DT.bfloat16)
            nc.vector.memset(ones_t[:], 1.0)

            def load_x(sp):
                s0 = sp * SP
                xb = xp.tile([P, KT, SP], DT.bfloat16, tag="xb", name="xb")
                for g in range(4):
                    nc.sync.dma_start(
                        out=xb[:, g * 8:(g + 1) * 8, :],
                        in_=xT[g * 8 * P:(g + 1) * 8 * P, s0:s0 + SP]
                            .rearrange("(k p) s -> p k s", p=P))
                return xb

            xb_pre = [load_x(0)]

            # ---- dequant, oi-major so o-chunk 0 is ready first ----
            wt = {}
            for oi, (o0, osz) in enumerate(O_CHUNKS):
                for kt in range(KT):
                    rows = slice(kt * P, (kt + 1) * P)
                    b0_t = dqp.tile([P, osz], DT.bfloat16, tag="pl_b0")
                    nc.sync.dma_start(out=b0_t[:], in_=b0_d[rows, o0:o0 + osz])
                    mp_t = dqp.tile([P, 4, osz], DT.uint8, tag="pl_mp")
                    nc.sync.dma_start(out=mp_t[:],
                                      in_=mpack_d[kt, :, :, o0:o0 + osz])
                    planes = [b0_t] + [mp_t[:, j, :] for j in range(4)]
                    sc_t = dqp.tile([P, osz], DT.bfloat16, tag="sc")
                    nc.sync.dma_start(out=sc_t[:], in_=scale[kt, :, o0:o0 + osz])
                    wt[(oi, kt)] = _dequant_chunk(nc, dq, wpool, planes, sc_t,
                                                  oi, kt, osz)

            def mm_block(sp, xb, ois, phase_mode):
                for half in range(2):
                    s0 = sp * SP + half * P
                    ps_ts = {oi: psp.tile([P, O_CHUNKS[oi][1]], DT.float32,
                                          tag="ps", name=f"ps_{sp}_{half}_{oi}")
                             for oi in ois}
                    sl = slice(half * P, (half + 1) * P)
                    for kt in range(KT):
                        for oi in ois:
                            nc.tensor.matmul(ps_ts[oi][:], xb[:, kt, sl],
                                             wt[(oi, kt)][:],
                                             start=(kt == 0),
                                             stop=(not phase_mode
                                                   and kt == KT - 1))
                    for oi in ois:
                        o0, osz = O_CHUNKS[oi]
                        out_t = op.tile([P, osz], DT.float32, tag="out")
                        if phase_mode:
                            nc.tensor.matmul(ps_ts[oi][:], ones_t[:1, :],
                                             bias_t[:1, o0:o0 + osz],
                                             start=False, stop=True)
                            nc.scalar.copy(out=out_t[:], in_=ps_ts[oi][:])
                        else:
                            nc.vector.tensor_tensor(out_t[:], ps_ts[oi][:],
                                                    biasw[:, o0:o0 + osz],
                                                    AluOpType.add)
                        nc.scalar.dma_start(out=y[s0:s0 + P, o0:o0 + osz],
                                            in_=out_t[:])

            # ---- phased token loop: A/B/C keep PE demand behind dequant.
            # x loads emitted one iteration ahead so prefetch outranks
            # same-iteration compute in scheduler priority. ----
            sched = ([(sp, [oi], True) for oi in range(3)
                      for sp in range(PHASE_SPS)]
                     + [(sp, [0, 1, 2], False)
                        for sp in range(PHASE_SPS, NSP)])
            for idx, (sp, ois, pm) in enumerate(sched):
                xb_cur = xb_pre.pop(0) if xb_pre else load_x(sp)
                if idx + 1 < len(sched):
                    xb_pre.append(load_x(sched[idx + 1][0]))
                mm_block(sp, xb_cur, ois, pm)

    nc.compile()
    return nc


def _get_nc():
    if 'nc' not in _CACHE:
        _CACHE['nc'] = _build()
    return _CACHE['nc']


def make_in_maps(x, weight_q, absmax, bias):
    x = np.asarray(x, dtype=np.float32)
    weight_q = np.asarray(weight_q)
    absmax = np.asarray(absmax, dtype=np.float32)
    bias = np.asarray(bias, dtype=np.float32)
    bf16 = ml_dtypes.bfloat16

    xT = np.ascontiguousarray(x.reshape(S_TOT, IN_F).T.astype(bf16))
    in_maps = []
    for c in range(N_CORES):
        sl = slice(c * O_LOC, (c + 1) * O_LOC)
        q_c = np.ascontiguousarray(weight_q[sl].T)      # [4096, 1376] int32
        hi = q_c >> 2
        mp = np.stack([((q_c >> 1) & 1), (hi == 1), (hi == 2), (hi == 3)],
                      axis=1).astype(np.uint8)        # [4096, 4, 1376]
        planes = {
            "b0": np.ascontiguousarray((q_c & 1).astype(bf16)),
            "mpack": np.ascontiguousarray(
                mp.reshape(KT, P, 4, O_LOC)),
        }
        am = absmax[sl]                                  # [O_LOC, 64]
        scale_c = np.ascontiguousarray(
            am.T.repeat(64, axis=0).astype(bf16)).reshape(KT, P, O_LOC)
        biasb_c = np.ascontiguousarray(bias[sl].astype(bf16).reshape(1, O_LOC))
        m = {"xT": xT, "scale": scale_c, "biasb": biasb_c}
        m.update(planes)
        in_maps.append(m)
    return in_maps


def kernel(x, weight_q, absmax, bias):
    nc = _get_nc()
    in_maps = make_in_maps(x, weight_q, absmax, bias)
    res = run_bass_kernel_spmd(nc, in_maps, core_ids=list(range(N_CORES)))
    y = np.concatenate([res.results[c]["y"] for c in range(N_CORES)], axis=1)
    return np.ascontiguousarray(y.reshape(4, 2048, OUT_F))

